# revision 18
# baseline (speedup 1.0000x reference)
"""Trainium2 Bass kernel for nn_BatchContrastLoss (InfoNCE-style contrastive loss).

Reference computation:
    sim[i,j]  = cos(que_i, ans_j)            (eps-guarded norms)
    logits    = sim / 0.07
    loss      = -mean_i(log_softmax(logits, axis=1)[i,i])

Sharding: data-parallel over rows of que across 8 NeuronCores. Each core
computes its [512, 4096] logits slab against the full ans batch and reduces
each row to a softmax denominator sum_j exp(logits[i,j]). The host takes
log + mean and subtracts the diagonal (the "all-reduce" of the hint).

Design (v7; baseline v1 was 101us, DVE/ScalarE-bound):
  - Row norms are folded into the fp8 quantization on the host: rows are
    normalized to unit length, scaled by 16 (keeps e4m3 mantissa well fed),
    and quantized. The device then needs NO norm computation at all: psum =
    (16*qhat)·(16*ahat) = 256*cos, and the exp drain folds 1/(256*gamma)
    into its free affine scale. The diagonal logits_ii are computed exactly
    on the host in f64 (O(B*D), negligible).
  - fp8e4m3 DoubleRow matmuls (K=256/instr, N=512 moving cols): measured
    216ns issue-to-issue warm => 128 MMs ~ 27.6us/core floor. LDWEIGHTS
    overlaps in the PE's reorder window.
  - Loop (g: 1024-col group, m: 128-row tile, c: bank, t: k-pair): one
    [128,1024] 2-bank PSUM tile per (g,m), drained in-place by a single
    ScalarE Exp with fused row-sum accumulation (~1.15us; ScalarE ~60% busy,
    off the critical path). The very last slab instead uses two independent
    [128,512] tiles so its first bank's drain overlaps its second bank's
    matmuls, shortening the tail.
  - DMA: all on the SP HWDGE ring in consumption order. Measured: ~630ns
    issue per dma_start, ~1.9us pipe fill, then ~0.45us/piece fixed +
    ~550GB/s marginal. Front is fine-grained (qm0 128KB, ans group 0 as two
    512KB halves) so real matmuls start ~10.9us; back is 1MB groups.
  - The PE clock gate (HAM) needs ~3.4us of *continuous* activity to
    unthrottle 1.2->2.4 GHz and any idle gap resets it. N_WARM dummy matmuls
    (N=256) bridge block start (~7.8us) to the first-data gate, and N_PATCH
    dummies bridge the c0->c1 half-gap of the first slab. A dummy Exp pulls
    the one-time ~2.7us activation table load off the critical path.
"""

import numpy as np

import concourse.bass as bass
import concourse.mybir as mybir
import concourse.tile as tile
from concourse import bacc
from concourse.bass_utils import run_bass_kernel_spmd

# Problem constants (self-contained; the harness provides only the inputs).
B = 4096  # rows of que_batch / ans_batch
D = 1024  # feature dim
NCORES = 8
NB = B // NCORES  # local que rows per core = 512
P = 128  # SBUF partitions
KT2 = 4  # k-pair tiles (each DoubleRow matmul contracts 256 dims)
NW = 512  # matmul moving width = one fp32 PSUM bank
G = 4  # ans column groups of 1024
MT = NB // P  # 4 row tiles of 128
GAMA = 0.07
EPS = 1e-8
SCALE = 16.0  # host quantization scale on unit rows
EXP_SCALE = 1.0 / (SCALE * SCALE * GAMA)  # psum -> logits
N_WARM = 14  # dummy matmuls bridging block start -> first-data gate
N_PATCH = 5  # dummy matmuls bridging slab 0's c0 -> c1 half arrival

F32 = mybir.dt.float32
FP8 = mybir.dt.float8e4  # e4m3
DR = mybir.MatmulPerfMode.DoubleRow
AF = mybir.ActivationFunctionType

OUTPUT_NAMES = ["s_out"]


def _build_program():
    nc = bacc.Bacc(
        "TRN2", target_bir_lowering=False, debug=False, num_devices=NCORES
    )

    # qPK[m, p, 2t+i, mm] = q16hat_fp8[local row 128m+mm, d=256t+128i+p]
    qPK = nc.dram_tensor("qPK", [MT, P, 2 * KT2, P], FP8, kind="ExternalInput").ap()
    # hPK[c, p, 2t+i, j] = a16hat_fp8[col 512c+j, d=256t+128i+p]  (group 0)
    hPK = nc.dram_tensor("hPK", [2, P, 2 * KT2, NW], FP8, kind="ExternalInput").ap()
    # aPK[g-1, p, 2t+i, j] = a16hat_fp8[col 1024g+j, d=256t+128i+p]  (g>=1)
    aPK = nc.dram_tensor(
        "aPK", [G - 1, P, 2 * KT2, 1024], FP8, kind="ExternalInput"
    ).ap()
    # s_out[p, 4g+m] = sum_{j in group g} exp(logits[row 128m+p, j]);
    # cols 15,16 are the two bank-halves of the last slab.
    s_out = nc.dram_tensor("s_out", [P, 17], F32, kind="ExternalOutput").ap()

    with tile.TileContext(nc) as tc:
        with (
            tc.tile_pool(name="persist", bufs=1) as persist,
            tc.tile_pool(name="psp", bufs=3, space="PSUM") as psp,
        ):
            _body(nc, persist, psp, qPK, hPK, aPK, s_out)

    nc.compile()
    return nc


def _body(nc, persist, psp, qPK, hPK, aPK, s_out):
    # ---- DMA front, all on the SP HWDGE ring in consumption order.
    qms = []

    def dma_q(m):
        qm = persist.tile([P, 2 * KT2, P], FP8, tag=f"qm_{m}", name=f"qm_{m}")
        nc.sync.dma_start(out=qm, in_=qPK[m])
        qms.append(qm)

    dma_q(0)
    hs = []
    for c in range(2):
        h = persist.tile([P, 2 * KT2, NW], FP8, tag=f"h_{c}", name=f"h_{c}")
        nc.sync.dma_start(out=h, in_=hPK[c])
        hs.append(h)
    for m in range(1, MT):
        dma_q(m)
    ags = []
    for g in range(1, G):
        a = persist.tile([P, 2 * KT2, 1024], FP8, tag=f"ag_{g}", name=f"ag_{g}")
        nc.sync.dma_start(out=a, in_=aPK[g - 1])
        ags.append(a)

    # ---- warmup: dummy Exp triggers the one-time activation table load;
    # dummy DoubleRow matmuls keep the PE busy with no gap from block start
    # until the qm[0]+h[0] gate so the HAM clock warms and stays warm.
    scr8 = persist.tile([P, 2, 256], FP8, tag="scr8")
    nc.gpsimd.memset(scr8, 0.0)
    scrf = persist.tile([P, 1], F32, tag="scrf")
    nc.gpsimd.memset(scrf, 0.0)
    dumo = persist.tile([P, 1], F32, tag="dumo")
    nc.scalar.activation(dumo, scrf, AF.Exp)

    ppw = psp.tile([P, 2 * NW], F32, tag="pp", name="pp_warm")

    def dummy_mms(n):
        for _ in range(n):
            nc.tensor.matmul(
                ppw[:, 0:256],
                lhsT=scr8[:, :, 0:P],
                rhs=scr8,
                start=True,
                stop=True,
                perf_mode=DR,
            )

    dummy_mms(N_WARM)

    # ---- main loop: 16 (g, m) slabs of [128 rows x 1024 cols]. Normal slabs
    # use one 2-bank PSUM tile + one Exp(accum) drain; the last slab uses two
    # independent 1-bank tiles so drain overlaps matmuls.
    s_sb_a = persist.tile([P, 12], F32, tag="s_sb_a")
    s_sb_b = persist.tile([P, 5], F32, tag="s_sb_b")

    def rhs_ap(g, c, t):
        if g == 0:
            return hs[c][:, 2 * t : 2 * t + 2, :]
        return ags[g - 1][:, 2 * t : 2 * t + 2, c * NW : (c + 1) * NW]

    for g in range(G):
        for m in range(MT):
            last = g == G - 1 and m == MT - 1
            if not last:
                pp = psp.tile([P, 2 * NW], F32, tag="pp", name=f"pp_{g}_{m}")
                for c in range(2):
                    for t in range(KT2):
                        nc.tensor.matmul(
                            pp[:, c * NW : (c + 1) * NW],
                            lhsT=qms[m][:, 2 * t : 2 * t + 2, :],
                            rhs=rhs_ap(g, c, t),
                            start=(t == 0),
                            stop=(t == KT2 - 1),
                            perf_mode=DR,
                        )
                    if g == 0 and m == 0 and c == 0:
                        # h_1 lands ~0.5us after slab 0 consumes h_0; keep
                        # the PE (and the HAM window) busy in between.
                        dummy_mms(N_PATCH)
                col = g * MT + m
                acc = (
                    s_sb_a[:, col : col + 1]
                    if col < 12
                    else s_sb_b[:, col - 12 : col - 11]
                )
                nc.scalar.activation(
                    pp, pp, AF.Exp, scale=float(EXP_SCALE), accum_out=acc
                )
            else:
                for c in range(2):
                    bk = psp.tile(
                        [P, NW], F32, tag=f"bk_{c}", name=f"bk_{c}", bufs=1
                    )
                    for t in range(KT2):
                        nc.tensor.matmul(
                            bk,
                            lhsT=qms[m][:, 2 * t : 2 * t + 2, :],
                            rhs=rhs_ap(g, c, t),
                            start=(t == 0),
                            stop=(t == KT2 - 1),
                            perf_mode=DR,
                        )
                    nc.scalar.activation(
                        bk,
                        bk,
                        AF.Exp,
                        scale=float(EXP_SCALE),
                        accum_out=s_sb_b[:, 3 + c : 4 + c],
                    )
        if g == G - 2:
            nc.sync.dma_start(out=s_out[:, 0:12], in_=s_sb_a)

    nc.sync.dma_start(out=s_out[:, 12:17], in_=s_sb_b)


_CACHE = {}


def _get_program():
    if "nc" not in _CACHE:
        _CACHE["nc"] = _build_program()
    return _CACHE["nc"]


def _make_in_maps(que, ans):
    """Normalize rows (folding the cosine norms into the quantization scale),
    quantize to fp8e4m3, and pack into the on-chip tile layouts. Also returns
    the exact host-computed diagonal logits."""
    fp8 = mybir.dt.np(FP8)
    que = np.asarray(que, dtype=np.float32)
    ans = np.asarray(ans, dtype=np.float32)

    qn = np.maximum(np.sqrt((que.astype(np.float64) ** 2).sum(1)), EPS)
    an = np.maximum(np.sqrt((ans.astype(np.float64) ** 2).sum(1)), EPS)
    q8 = (que * (SCALE / qn[:, None]).astype(np.float32)).astype(fp8)
    a8 = (ans * (SCALE / an[:, None]).astype(np.float32)).astype(fp8)

    # diag logits (exact, f64): cos(q_i, a_i) / gamma
    diag = (que.astype(np.float64) * ans.astype(np.float64)).sum(1) / (
        qn * an * GAMA
    )

    # full pack [g, p, 2t+i, j] = a8[1024g+j, 256t+128i+p] (shared by cores)
    ap_all = np.ascontiguousarray(
        a8.reshape(G, 1024, KT2, 2, P).transpose(0, 4, 2, 3, 1)
    ).reshape(G, P, 2 * KT2, 1024)
    # group 0 split into 512-col halves: hPK[c, p, 2t+i, j]
    hPK = np.ascontiguousarray(
        ap_all[0].reshape(P, 2 * KT2, 2, NW).transpose(2, 0, 1, 3)
    )
    aPK = ap_all[1:]

    in_maps = []
    for c in range(NCORES):
        qc = q8[c * NB : (c + 1) * NB]  # [512, 1024]
        # qPK[m, p, 2t+i, mm] = qc[128m+mm, 256t+128i+p]
        qPK = np.ascontiguousarray(
            qc.reshape(MT, P, KT2, 2, P).transpose(0, 4, 2, 3, 1)
        ).reshape(MT, P, 2 * KT2, P)
        in_maps.append({"qPK": qPK, "hPK": hPK, "aPK": aPK})
    return in_maps, diag


def _finish(results, diag):
    # s_out[p, 4g+m]: per-group partial softmax denominators (cols 15+16 are
    # the two halves of the last slab).
    denoms = []
    for r in results:
        so = np.asarray(r["s_out"])  # [P, 17]
        s16 = np.concatenate([so[:, :15], (so[:, 15] + so[:, 16])[:, None]], axis=1)
        s = s16.reshape(P, G, MT).sum(axis=1)  # [p, m]
        denoms.append(s.T.reshape(-1))  # local row order m*128+p
    denom = np.concatenate(denoms)  # [B]
    lse = np.log(denom.astype(np.float64))
    loss = np.float32(np.mean(lse - diag))
    return np.array([loss], dtype=np.float32)


def kernel(que_batch, ans_batch):
    nc = _get_program()
    in_maps, diag = _make_in_maps(np.asarray(que_batch), np.asarray(ans_batch))
    res = run_bass_kernel_spmd(nc, in_maps, list(range(NCORES)))
    return _finish(res.results, diag)


if __name__ == "__main__":
    rng = np.random.default_rng(0)
    q = rng.standard_normal((B, D), dtype=np.float32)
    a = rng.standard_normal((B, D), dtype=np.float32)
    print(kernel(q, a))


# revision 19
# speedup vs baseline: 1.0135x; 1.0135x over previous
"""Trainium2 Bass kernel for nn_BatchContrastLoss (InfoNCE-style contrastive loss).

Reference computation:
    sim[i,j]  = cos(que_i, ans_j)            (eps-guarded norms)
    logits    = sim / 0.07
    loss      = -mean_i(log_softmax(logits, axis=1)[i,i])

Sharding: data-parallel over rows of que across 8 NeuronCores. Each core
computes its [512, 4096] logits slab against the full ans batch and reduces
each row to a softmax denominator sum_j exp(logits[i,j]). The host takes
log + mean and subtracts the diagonal (the "all-reduce" of the hint).

Design (v8; baseline v1 was 101us, DVE/ScalarE-bound):
  - Row norms are folded into the fp8 quantization on the host: rows are
    normalized to unit length, scaled by 16 (keeps e4m3 mantissa well fed),
    and quantized. The device then needs NO norm computation at all: psum =
    (16*qhat)·(16*ahat) = 256*cos, and the exp drain folds 1/(256*gamma)
    into its free affine scale. The diagonal logits_ii are computed exactly
    on the host in f64 (O(B*D), negligible).
  - fp8e4m3 DoubleRow matmuls (K=256/instr, N=512 moving cols): measured
    216ns issue-to-issue warm => 128 MMs ~ 27.6us/core floor. LDWEIGHTS
    overlaps in the PE's reorder window.
  - Loop (g: 1024-col group, m: 128-row tile, c: bank, t: k-pair): one
    [128,1024] 2-bank PSUM tile per (g,m), drained in-place by a single
    ScalarE Exp with fused row-sum accumulation ((1024+352)/1.2 ~ 1.15us;
    16 total => ScalarE ~60% busy, off the critical path). Splitting drains
    finer was measured SLOWER (the +352-cycle fixed cost dominates).
  - DMA: all input pieces on the SP HWDGE ring in consumption order.
    Measured: ~630ns issue per dma_start, and the pipe delivers the FIRST
    ~1MB at only ~230GB/s (cold) before reaching ~400GB/s, so the first-data
    gate sits at ~13.3us no matter how the front is sliced (finer slicing
    was measured net-slower). que rides as 4x128KB per-m-tile pieces with
    ans groups of 1MB behind.
  - The PE clock gate (HAM) needs ~3.4us of *continuous* activity aligned
    to its free-running window to unthrottle 1.2->2.4 GHz, and any idle gap
    resets it. N_WARM=28 dummy matmuls (N=256) bridge the PE from block
    start (~7.8us) to the DMA gate with no gap: measured zero-gap stream
    with every real matmul at the warm 216ns cadence. A dummy Exp pulls the
    one-time ~2.7us activation table load off the critical path.
  - Outputs: 12 of the 16 accumulator columns ship out mid-kernel; the
    final 4-column DMA is issued from the ScalarE HWDGE ring so it chains
    directly behind the last accumulator read.
"""

import numpy as np

import concourse.bass as bass
import concourse.mybir as mybir
import concourse.tile as tile
from concourse import bacc
from concourse.bass_utils import run_bass_kernel_spmd

# Problem constants (self-contained; the harness provides only the inputs).
B = 4096  # rows of que_batch / ans_batch
D = 1024  # feature dim
NCORES = 8
NB = B // NCORES  # local que rows per core = 512
P = 128  # SBUF partitions
KT2 = 4  # k-pair tiles (each DoubleRow matmul contracts 256 dims)
NW = 512  # matmul moving width = one fp32 PSUM bank
G = 4  # ans column groups of 1024
MT = NB // P  # 4 row tiles of 128
GAMA = 0.07
EPS = 1e-8
SCALE = 16.0  # host quantization scale on unit rows
EXP_SCALE = 1.0 / (SCALE * SCALE * GAMA)  # psum -> logits
N_WARM = 28  # dummy matmuls bridging block start -> first-data gate

F32 = mybir.dt.float32
FP8 = mybir.dt.float8e4  # e4m3
DR = mybir.MatmulPerfMode.DoubleRow
AF = mybir.ActivationFunctionType

OUTPUT_NAMES = ["s_out"]


def _build_program():
    nc = bacc.Bacc(
        "TRN2", target_bir_lowering=False, debug=False, num_devices=NCORES
    )

    # qPK[m, p, 2t+i, mm] = q16hat_fp8[local row 128m+mm, d=256t+128i+p]
    qPK = nc.dram_tensor("qPK", [MT, P, 2 * KT2, P], FP8, kind="ExternalInput").ap()
    # aPK[g, p, 2t+i, j] = a16hat_fp8[col 1024g+j, d=256t+128i+p]
    aPK = nc.dram_tensor("aPK", [G, P, 2 * KT2, 1024], FP8, kind="ExternalInput").ap()
    # s_out[p, 4g+m] = sum_{j in group g} exp(logits[row 128m+p, j])
    s_out = nc.dram_tensor("s_out", [P, G * MT], F32, kind="ExternalOutput").ap()

    with tile.TileContext(nc) as tc:
        with (
            tc.tile_pool(name="persist", bufs=1) as persist,
            tc.tile_pool(name="psp", bufs=4, space="PSUM") as psp,
        ):
            _body(nc, persist, psp, qPK, aPK, s_out)

    nc.compile()
    return nc


def _body(nc, persist, psp, qPK, aPK, s_out):
    # ---- DMA front, all on the SP HWDGE ring in consumption order.
    qms = []
    ags = []

    def dma_q(m):
        qm = persist.tile([P, 2 * KT2, P], FP8, tag=f"qm_{m}", name=f"qm_{m}")
        nc.sync.dma_start(out=qm, in_=qPK[m])
        qms.append(qm)

    def dma_a(g):
        a = persist.tile([P, 2 * KT2, 1024], FP8, tag=f"ag_{g}", name=f"ag_{g}")
        nc.sync.dma_start(out=a, in_=aPK[g])
        ags.append(a)

    dma_q(0)
    dma_a(0)
    for m in range(1, MT):
        dma_q(m)
    for g in range(1, G):
        dma_a(g)

    # ---- warmup: dummy Exp triggers the one-time activation table load;
    # dummy DoubleRow matmuls keep the PE busy with no gap from block start
    # until the qm[0]+ag[0] gate, so the HAM clock is warm for every real
    # matmul. All on zeroed scratch, off to the side.
    scr8 = persist.tile([P, 2, 256], FP8, tag="scr8")
    nc.gpsimd.memset(scr8, 0.0)
    scrf = persist.tile([P, 1], F32, tag="scrf")
    nc.gpsimd.memset(scrf, 0.0)
    dumo = persist.tile([P, 1], F32, tag="dumo")
    nc.scalar.activation(dumo, scrf, AF.Exp)

    ppw = psp.tile([P, 2 * NW], F32, tag="pp", name="pp_warm")
    for w in range(N_WARM):
        nc.tensor.matmul(
            ppw[:, 0:256],
            lhsT=scr8[:, :, 0:P],
            rhs=scr8,
            start=True,
            stop=True,
            perf_mode=DR,
        )

    # ---- main loop: 16 (g, m) slabs of [128 rows x 1024 cols], each one
    # 2-bank PSUM tile built by 8 DoubleRow matmuls, drained in-place by a
    # single Exp with fused row-sum accumulation. The first 12 accumulator
    # columns ship out early so only a tiny DMA trails the last drain.
    s_sb_a = persist.tile([P, 12], F32, tag="s_sb_a")
    s_sb_b = persist.tile([P, 4], F32, tag="s_sb_b")
    for g in range(G):
        for m in range(MT):
            pp = psp.tile([P, 2 * NW], F32, tag="pp", name=f"pp_{g}_{m}")
            for c in range(2):
                for t in range(KT2):
                    rhs = ags[g][:, 2 * t : 2 * t + 2, c * NW : (c + 1) * NW]
                    nc.tensor.matmul(
                        pp[:, c * NW : (c + 1) * NW],
                        lhsT=qms[m][:, 2 * t : 2 * t + 2, :],
                        rhs=rhs,
                        start=(t == 0),
                        stop=(t == KT2 - 1),
                        perf_mode=DR,
                    )
            col = g * MT + m
            acc = (
                s_sb_a[:, col : col + 1]
                if col < 12
                else s_sb_b[:, col - 12 : col - 11]
            )
            nc.scalar.activation(
                pp,
                pp,
                AF.Exp,
                scale=float(EXP_SCALE),
                accum_out=acc,
            )
        if g == G - 2:
            nc.sync.dma_start(out=s_out[:, 0:12], in_=s_sb_a)

    nc.scalar.dma_start(out=s_out[:, 12:16], in_=s_sb_b)


_CACHE = {}


def _get_program():
    if "nc" not in _CACHE:
        _CACHE["nc"] = _build_program()
    return _CACHE["nc"]


def _make_in_maps(que, ans):
    """Normalize rows (folding the cosine norms into the quantization scale),
    quantize to fp8e4m3, and pack into the on-chip tile layouts. Also returns
    the exact host-computed diagonal logits."""
    fp8 = mybir.dt.np(FP8)
    que = np.asarray(que, dtype=np.float32)
    ans = np.asarray(ans, dtype=np.float32)

    qn = np.maximum(np.sqrt((que.astype(np.float64) ** 2).sum(1)), EPS)
    an = np.maximum(np.sqrt((ans.astype(np.float64) ** 2).sum(1)), EPS)
    q8 = (que * (SCALE / qn[:, None]).astype(np.float32)).astype(fp8)
    a8 = (ans * (SCALE / an[:, None]).astype(np.float32)).astype(fp8)

    # diag logits (exact, f64): cos(q_i, a_i) / gamma
    diag = (que.astype(np.float64) * ans.astype(np.float64)).sum(1) / (
        qn * an * GAMA
    )

    # aPK[g, p, 2t+i, j] = a8[1024g+j, 256t+128i+p]  (shared by all cores)
    aPK = np.ascontiguousarray(
        a8.reshape(G, 1024, KT2, 2, P).transpose(0, 4, 2, 3, 1)
    ).reshape(G, P, 2 * KT2, 1024)

    in_maps = []
    for c in range(NCORES):
        qc = q8[c * NB : (c + 1) * NB]  # [512, 1024]
        # qPK[m, p, 2t+i, mm] = qc[128m+mm, 256t+128i+p]
        qPK = np.ascontiguousarray(
            qc.reshape(MT, P, KT2, 2, P).transpose(0, 4, 2, 3, 1)
        ).reshape(MT, P, 2 * KT2, P)
        in_maps.append({"qPK": qPK, "aPK": aPK})
    return in_maps, diag


def _finish(results, diag):
    # s_out[p, 4g+m]: per-group partial softmax denominators.
    denoms = []
    for r in results:
        s = np.asarray(r["s_out"]).reshape(P, G, MT).sum(axis=1)  # [p, m]
        denoms.append(s.T.reshape(-1))  # local row order m*128+p
    denom = np.concatenate(denoms)  # [B]
    lse = np.log(denom.astype(np.float64))
    loss = np.float32(np.mean(lse - diag))
    return np.array([loss], dtype=np.float32)


def kernel(que_batch, ans_batch):
    nc = _get_program()
    in_maps, diag = _make_in_maps(np.asarray(que_batch), np.asarray(ans_batch))
    res = run_bass_kernel_spmd(nc, in_maps, list(range(NCORES)))
    return _finish(res.results, diag)


if __name__ == "__main__":
    rng = np.random.default_rng(0)
    q = rng.standard_normal((B, D), dtype=np.float32)
    a = rng.standard_normal((B, D), dtype=np.float32)
    print(kernel(q, a))


# revision 21
# speedup vs baseline: 1.0380x; 1.0242x over previous
"""Trainium2 Bass kernel for nn_BatchContrastLoss (InfoNCE-style contrastive loss).

Reference computation:
    sim[i,j]  = cos(que_i, ans_j)            (eps-guarded norms)
    logits    = sim / 0.07
    loss      = -mean_i(log_softmax(logits, axis=1)[i,i])

Sharding: data-parallel over rows of que across 8 NeuronCores. Each core
computes its [512, 4096] logits slab against the full ans batch and reduces
each row to a softmax denominator sum_j exp(logits[i,j]). The host takes
log + mean and subtracts the diagonal (the "all-reduce" of the hint).

Design (v8; baseline v1 was 101us, DVE/ScalarE-bound):
  - Row norms are folded into the fp8 quantization on the host: rows are
    normalized to unit length, scaled by 16 (keeps e4m3 mantissa well fed),
    and quantized. The device then needs NO norm computation at all: psum =
    (16*qhat)·(16*ahat) = 256*cos, and the exp drain folds 1/(256*gamma)
    into its free affine scale. The diagonal logits_ii are computed exactly
    on the host in f64 (O(B*D), negligible).
  - fp8e4m3 DoubleRow matmuls (K=256/instr, N=512 moving cols): measured
    216ns issue-to-issue warm => 128 MMs ~ 27.6us/core floor. LDWEIGHTS
    overlaps in the PE's reorder window.
  - Loop (g: 1024-col group, m: 128-row tile, c: bank, t: k-pair): one
    [128,1024] 2-bank PSUM tile per (g,m), drained in-place by a single
    ScalarE Exp with fused row-sum accumulation ((1024+352)/1.2 ~ 1.15us;
    16 total => ScalarE ~60% busy, off the critical path). Splitting drains
    finer was measured SLOWER (the +352-cycle fixed cost dominates).
  - DMA: all input pieces on the SP HWDGE ring in consumption order.
    Measured: ~630ns issue per dma_start, and the pipe delivers the FIRST
    ~1MB at only ~230GB/s (cold) before reaching ~400GB/s, so the first-data
    gate sits at ~13.3us no matter how the front is sliced (finer slicing
    was measured net-slower). que rides as 4x128KB per-m-tile pieces with
    ans groups of 1MB behind.
  - The PE clock gate (HAM) needs ~3.4us of *continuous* activity aligned
    to its free-running window to unthrottle 1.2->2.4 GHz, and any idle gap
    resets it. N_WARM=28 dummy matmuls (N=256) bridge the PE from block
    start (~7.8us) to the DMA gate with no gap: measured zero-gap stream
    with every real matmul at the warm 216ns cadence. A dummy Exp pulls the
    one-time ~2.7us activation table load off the critical path.
  - Outputs: 12 of the 16 accumulator columns ship out mid-kernel so only a
    tiny 2KB DMA trails the last drain.
"""

import numpy as np

import concourse.bass as bass
import concourse.mybir as mybir
import concourse.tile as tile
from concourse import bacc
from concourse.bass_utils import run_bass_kernel_spmd

# Problem constants (self-contained; the harness provides only the inputs).
B = 4096  # rows of que_batch / ans_batch
D = 1024  # feature dim
NCORES = 8
NB = B // NCORES  # local que rows per core = 512
P = 128  # SBUF partitions
KT2 = 4  # k-pair tiles (each DoubleRow matmul contracts 256 dims)
NW = 512  # matmul moving width = one fp32 PSUM bank
G = 4  # ans column groups of 1024
MT = NB // P  # 4 row tiles of 128
GAMA = 0.07
EPS = 1e-8
SCALE = 16.0  # host quantization scale on unit rows
EXP_SCALE = 1.0 / (SCALE * SCALE * GAMA)  # psum -> logits
N_WARM = 28  # dummy matmuls bridging block start -> first-data gate

F32 = mybir.dt.float32
FP8 = mybir.dt.float8e4  # e4m3
DR = mybir.MatmulPerfMode.DoubleRow
AF = mybir.ActivationFunctionType

OUTPUT_NAMES = ["s_out"]


def _build_program():
    nc = bacc.Bacc(
        "TRN2", target_bir_lowering=False, debug=False, num_devices=NCORES
    )

    # qPK[m, p, 2t+i, mm] = q16hat_fp8[local row 128m+mm, d=256t+128i+p]
    qPK = nc.dram_tensor("qPK", [MT, P, 2 * KT2, P], FP8, kind="ExternalInput").ap()
    # aPK[g, p, 2t+i, j] = a16hat_fp8[col 1024g+j, d=256t+128i+p]
    aPK = nc.dram_tensor("aPK", [G, P, 2 * KT2, 1024], FP8, kind="ExternalInput").ap()
    # s_out[p, 4g+m] = sum_{j in group g} exp(logits[row 128m+p, j])
    s_out = nc.dram_tensor("s_out", [P, G * MT], F32, kind="ExternalOutput").ap()

    with tile.TileContext(nc) as tc:
        with (
            tc.tile_pool(name="persist", bufs=1) as persist,
            tc.tile_pool(name="psp", bufs=4, space="PSUM") as psp,
        ):
            _body(nc, persist, psp, qPK, aPK, s_out)

    nc.compile()
    return nc


def _body(nc, persist, psp, qPK, aPK, s_out):
    # ---- DMA front, all on the SP HWDGE ring in consumption order.
    qms = []
    ags = []

    def dma_q(m):
        qm = persist.tile([P, 2 * KT2, P], FP8, tag=f"qm_{m}", name=f"qm_{m}")
        nc.sync.dma_start(out=qm, in_=qPK[m])
        qms.append(qm)

    def dma_a(g):
        a = persist.tile([P, 2 * KT2, 1024], FP8, tag=f"ag_{g}", name=f"ag_{g}")
        nc.sync.dma_start(out=a, in_=aPK[g])
        ags.append(a)

    dma_q(0)
    dma_a(0)
    for m in range(1, MT):
        dma_q(m)
    for g in range(1, G):
        dma_a(g)

    # ---- warmup: dummy Exp triggers the one-time activation table load;
    # dummy DoubleRow matmuls keep the PE busy with no gap from block start
    # until the qm[0]+ag[0] gate, so the HAM clock is warm for every real
    # matmul. All on zeroed scratch, off to the side.
    scr8 = persist.tile([P, 2, 256], FP8, tag="scr8")
    nc.gpsimd.memset(scr8, 0.0)
    scrf = persist.tile([P, 1], F32, tag="scrf")
    nc.gpsimd.memset(scrf, 0.0)
    dumo = persist.tile([P, 1], F32, tag="dumo")
    nc.scalar.activation(dumo, scrf, AF.Exp)

    ppw = psp.tile([P, 2 * NW], F32, tag="pp", name="pp_warm")
    for w in range(N_WARM):
        nc.tensor.matmul(
            ppw[:, 0:256],
            lhsT=scr8[:, :, 0:P],
            rhs=scr8,
            start=True,
            stop=True,
            perf_mode=DR,
        )

    # ---- main loop: 16 (g, m) slabs of [128 rows x 1024 cols], each one
    # 2-bank PSUM tile built by 8 DoubleRow matmuls, drained in-place by a
    # single Exp with fused row-sum accumulation. The first 12 accumulator
    # columns ship out early so only a tiny DMA trails the last drain.
    s_sb_a = persist.tile([P, 12], F32, tag="s_sb_a")
    s_sb_b = persist.tile([P, 4], F32, tag="s_sb_b")
    for g in range(G):
        for m in range(MT):
            pp = psp.tile([P, 2 * NW], F32, tag="pp", name=f"pp_{g}_{m}")
            for c in range(2):
                for t in range(KT2):
                    rhs = ags[g][:, 2 * t : 2 * t + 2, c * NW : (c + 1) * NW]
                    nc.tensor.matmul(
                        pp[:, c * NW : (c + 1) * NW],
                        lhsT=qms[m][:, 2 * t : 2 * t + 2, :],
                        rhs=rhs,
                        start=(t == 0),
                        stop=(t == KT2 - 1),
                        perf_mode=DR,
                    )
            col = g * MT + m
            acc = (
                s_sb_a[:, col : col + 1]
                if col < 12
                else s_sb_b[:, col - 12 : col - 11]
            )
            nc.scalar.activation(
                pp,
                pp,
                AF.Exp,
                scale=float(EXP_SCALE),
                accum_out=acc,
            )
        if g == G - 2:
            nc.sync.dma_start(out=s_out[:, 0:12], in_=s_sb_a)

    nc.sync.dma_start(out=s_out[:, 12:16], in_=s_sb_b)


_CACHE = {}


def _get_program():
    if "nc" not in _CACHE:
        _CACHE["nc"] = _build_program()
    return _CACHE["nc"]


def _make_in_maps(que, ans):
    """Normalize rows (folding the cosine norms into the quantization scale),
    quantize to fp8e4m3, and pack into the on-chip tile layouts. Also returns
    the exact host-computed diagonal logits."""
    fp8 = mybir.dt.np(FP8)
    que = np.asarray(que, dtype=np.float32)
    ans = np.asarray(ans, dtype=np.float32)

    qn = np.maximum(np.sqrt((que.astype(np.float64) ** 2).sum(1)), EPS)
    an = np.maximum(np.sqrt((ans.astype(np.float64) ** 2).sum(1)), EPS)
    q8 = (que * (SCALE / qn[:, None]).astype(np.float32)).astype(fp8)
    a8 = (ans * (SCALE / an[:, None]).astype(np.float32)).astype(fp8)

    # diag logits (exact, f64): cos(q_i, a_i) / gamma
    diag = (que.astype(np.float64) * ans.astype(np.float64)).sum(1) / (
        qn * an * GAMA
    )

    # aPK[g, p, 2t+i, j] = a8[1024g+j, 256t+128i+p]  (shared by all cores)
    aPK = np.ascontiguousarray(
        a8.reshape(G, 1024, KT2, 2, P).transpose(0, 4, 2, 3, 1)
    ).reshape(G, P, 2 * KT2, 1024)

    in_maps = []
    for c in range(NCORES):
        qc = q8[c * NB : (c + 1) * NB]  # [512, 1024]
        # qPK[m, p, 2t+i, mm] = qc[128m+mm, 256t+128i+p]
        qPK = np.ascontiguousarray(
            qc.reshape(MT, P, KT2, 2, P).transpose(0, 4, 2, 3, 1)
        ).reshape(MT, P, 2 * KT2, P)
        in_maps.append({"qPK": qPK, "aPK": aPK})
    return in_maps, diag


def _finish(results, diag):
    # s_out[p, 4g+m]: per-group partial softmax denominators.
    denoms = []
    for r in results:
        s = np.asarray(r["s_out"]).reshape(P, G, MT).sum(axis=1)  # [p, m]
        denoms.append(s.T.reshape(-1))  # local row order m*128+p
    denom = np.concatenate(denoms)  # [B]
    lse = np.log(denom.astype(np.float64))
    loss = np.float32(np.mean(lse - diag))
    return np.array([loss], dtype=np.float32)


def kernel(que_batch, ans_batch):
    nc = _get_program()
    in_maps, diag = _make_in_maps(np.asarray(que_batch), np.asarray(ans_batch))
    res = run_bass_kernel_spmd(nc, in_maps, list(range(NCORES)))
    return _finish(res.results, diag)


if __name__ == "__main__":
    rng = np.random.default_rng(0)
    q = rng.standard_normal((B, D), dtype=np.float32)
    a = rng.standard_normal((B, D), dtype=np.float32)
    print(kernel(q, a))


# revision 22
# speedup vs baseline: 1.0420x; 1.0038x over previous
"""Trainium2 Bass kernel for nn_BatchContrastLoss (InfoNCE-style contrastive loss).

Reference computation:
    sim[i,j]  = cos(que_i, ans_j)            (eps-guarded norms)
    logits    = sim / 0.07
    loss      = -mean_i(log_softmax(logits, axis=1)[i,i])

Sharding: data-parallel over rows of que across 8 NeuronCores. Each core
computes its [512, 4096] logits slab against the full ans batch and reduces
each row to a softmax denominator sum_j exp(logits[i,j]). The host takes
log + mean and subtracts the diagonal (the "all-reduce" of the hint).

Design (v9; baseline v1 was 101us, DVE/ScalarE-bound; v6 checkpoint 46.5us):
  - Row norms are folded into the fp8 quantization on the host: rows are
    normalized to unit length, scaled by 16, and quantized. The device needs
    NO norm computation: psum = (16*qhat)·(16*ahat) = 256*cos and the exp
    drain folds 1/(256*gamma) into its free affine scale. The diagonal
    logits_ii is computed exactly on the host in f64 (O(B*D), negligible).
  - fp8e4m3 DoubleRow matmuls (K=256/instr, N=512): measured 216ns
    issue-to-issue warm => 128 MMs ~ 27.6us/core floor.
  - Stream-end lower bound = max over DMA pieces of (arrival + MM work that
    must follow it). The ans columns are therefore grouped unevenly
    [512, 1024, 1024, 1024, 512]: the first 512-col group (512KB) lands
    ~2.6us before a 1MB group would, the 512-col tail group shortens the
    final drain, and group 1 ships as two 512KB halves (compute still does
    one [128,1024] slab; the halves just live in two SBUF tiles).
  - Slabs: [128 x W] PSUM tiles (2 banks allocated; W/512 banks used),
    8 DoubleRow matmuls per 1024-wide slab, drained in-place by one ScalarE
    Exp with fused row-sum accumulation ((W+352)/1.2 ns). 20 drains total
    ~25us ScalarE, still under the PE's 27.6us.
  - DMA: all on the SP HWDGE ring in consumption order (~630ns issue each,
    ~1.9us pipe fill, ~0.45us/piece + ~550GB/s marginal; the first ~1MB
    moves at only ~230GB/s, which is what makes the small first group pay).
  - The PE clock gate (HAM) needs ~3.4us of continuous activity to
    unthrottle 1.2->2.4 GHz and any idle gap before that resets it; N_WARM
    dummy matmuls bridge block start (~7.8us) to the first-data gate. Gaps
    after warm-up only cost their own length. A dummy Exp pulls the one-time
    ~2.7us activation table load off the critical path.
  - Outputs: 16 of the 20 accumulator columns ship out mid-kernel; only a
    tiny DMA trails the last (512-wide, cheaper) drain.
"""

import numpy as np

import concourse.bass as bass
import concourse.mybir as mybir
import concourse.tile as tile
from concourse import bacc
from concourse.bass_utils import run_bass_kernel_spmd

# Problem constants (self-contained; the harness provides only the inputs).
B = 4096  # rows of que_batch / ans_batch
D = 1024  # feature dim
NCORES = 8
NB = B // NCORES  # local que rows per core = 512
P = 128  # SBUF partitions
KT2 = 4  # k-pair tiles (each DoubleRow matmul contracts 256 dims)
NW = 512  # matmul moving width = one fp32 PSUM bank
MT = NB // P  # 4 row tiles of 128
GAMA = 0.07
EPS = 1e-8
SCALE = 16.0  # host quantization scale on unit rows
EXP_SCALE = 1.0 / (SCALE * SCALE * GAMA)  # psum -> logits
N_WARM = 15  # dummy matmuls bridging block start -> first-data gate

# ans column groups (uneven): widths in 512-col banks.
GW = [1, 2, 2, 2, 1]  # 512, 1024, 1024, 1024, 512 columns
NSLAB = len(GW) * MT  # 20 slabs / accumulator columns

F32 = mybir.dt.float32
FP8 = mybir.dt.float8e4  # e4m3
DR = mybir.MatmulPerfMode.DoubleRow
AF = mybir.ActivationFunctionType

OUTPUT_NAMES = ["s_out"]


def _build_program():
    nc = bacc.Bacc(
        "TRN2", target_bir_lowering=False, debug=False, num_devices=NCORES
    )

    # qPK[m, p, 2t+i, mm] = q16hat_fp8[local row 128m+mm, d=256t+128i+p]
    qPK = nc.dram_tensor("qPK", [MT, P, 2 * KT2, P], FP8, kind="ExternalInput").ap()
    # 512-col pieces: aS[k, p, 2t+i, j]; k: 0=grp0, 1=grp1 c0, 2=grp1 c1, 3=grp4
    aS = nc.dram_tensor("aS", [4, P, 2 * KT2, NW], FP8, kind="ExternalInput").ap()
    # 1024-col groups 2 and 3: aF[k, p, 2t+i, j]
    aF = nc.dram_tensor("aF", [2, P, 2 * KT2, 1024], FP8, kind="ExternalInput").ap()
    # s_out[p, 4*grp+m] = sum_{j in grp} exp(logits[row 128m+p, j])
    s_out = nc.dram_tensor("s_out", [P, NSLAB], F32, kind="ExternalOutput").ap()

    with tile.TileContext(nc) as tc:
        with (
            tc.tile_pool(name="persist", bufs=1) as persist,
            tc.tile_pool(name="psp", bufs=4, space="PSUM") as psp,
        ):
            _body(nc, persist, psp, qPK, aS, aF, s_out)

    nc.compile()
    return nc


def _body(nc, persist, psp, qPK, aS, aF, s_out):
    # ---- DMA front, all on the SP HWDGE ring in consumption order.
    qms = []

    def dma_q(m):
        qm = persist.tile([P, 2 * KT2, P], FP8, tag=f"qm_{m}", name=f"qm_{m}")
        nc.sync.dma_start(out=qm, in_=qPK[m])
        qms.append(qm)

    def dma_s(k):
        t = persist.tile([P, 2 * KT2, NW], FP8, tag=f"as_{k}", name=f"as_{k}")
        nc.sync.dma_start(out=t, in_=aS[k])
        return t

    def dma_f(k):
        t = persist.tile([P, 2 * KT2, 1024], FP8, tag=f"af_{k}", name=f"af_{k}")
        nc.sync.dma_start(out=t, in_=aF[k])
        return t

    dma_q(0)
    a0 = dma_s(0)
    for m in range(1, MT):
        dma_q(m)
    g1a = dma_s(1)
    g1b = dma_s(2)
    g2 = dma_f(0)
    g3 = dma_f(1)
    a4 = dma_s(3)

    # rhs AP for (group, c-half, t)
    def rhs_ap(grp, c, t):
        sl = slice(2 * t, 2 * t + 2)
        if grp == 0:
            return a0[:, sl, :]
        if grp == 1:
            return (g1a if c == 0 else g1b)[:, sl, :]
        if grp == 4:
            return a4[:, sl, :]
        f = g2 if grp == 2 else g3
        return f[:, sl, c * NW : (c + 1) * NW]

    # ---- warmup: dummy Exp triggers the one-time activation table load;
    # dummy DoubleRow matmuls keep the PE busy with no gap from block start
    # until the qm[0]+grp0 gate, so the HAM clock is warm for every real
    # matmul. All on zeroed scratch, off to the side.
    scr8 = persist.tile([P, 2, 256], FP8, tag="scr8")
    nc.gpsimd.memset(scr8, 0.0)
    scrf = persist.tile([P, 1], F32, tag="scrf")
    nc.gpsimd.memset(scrf, 0.0)
    dumo = persist.tile([P, 1], F32, tag="dumo")
    nc.scalar.activation(dumo, scrf, AF.Exp)

    ppw = psp.tile([P, 2 * NW], F32, tag="pp", name="pp_warm")
    for w in range(N_WARM):
        nc.tensor.matmul(
            ppw[:, 0:256],
            lhsT=scr8[:, :, 0:P],
            rhs=scr8,
            start=True,
            stop=True,
            perf_mode=DR,
        )

    # ---- main loop: 20 (grp, m) slabs of [128 rows x 512|1024 cols], each a
    # PSUM tile (1 or 2 banks used) built by DoubleRow matmuls and drained
    # in-place by a single Exp with fused row-sum accumulation. The first 16
    # accumulator columns ship out early.
    s_sb_a = persist.tile([P, 16], F32, tag="s_sb_a")
    s_sb_b = persist.tile([P, 4], F32, tag="s_sb_b")
    col = 0
    for grp, w in enumerate(GW):
        for m in range(MT):
            pp = psp.tile([P, 2 * NW], F32, tag="pp", name=f"pp_{grp}_{m}")
            for c in range(w):
                for t in range(KT2):
                    nc.tensor.matmul(
                        pp[:, c * NW : (c + 1) * NW],
                        lhsT=qms[m][:, 2 * t : 2 * t + 2, :],
                        rhs=rhs_ap(grp, c, t),
                        start=(t == 0),
                        stop=(t == KT2 - 1),
                        perf_mode=DR,
                    )
            acc = (
                s_sb_a[:, col : col + 1]
                if col < 16
                else s_sb_b[:, col - 16 : col - 15]
            )
            nc.scalar.activation(
                pp[:, 0 : w * NW],
                pp[:, 0 : w * NW],
                AF.Exp,
                scale=float(EXP_SCALE),
                accum_out=acc,
            )
            col += 1
        if grp == len(GW) - 2:
            nc.sync.dma_start(out=s_out[:, 0:16], in_=s_sb_a)

    nc.sync.dma_start(out=s_out[:, 16:NSLAB], in_=s_sb_b)


_CACHE = {}


def _get_program():
    if "nc" not in _CACHE:
        _CACHE["nc"] = _build_program()
    return _CACHE["nc"]


def _pack_cols(a8, lo, hi):
    """[p, 2t+i, j] packing of ans columns [lo, hi)."""
    w = hi - lo
    return np.ascontiguousarray(
        a8[lo:hi].reshape(w, KT2, 2, P).transpose(3, 1, 2, 0)
    ).reshape(P, 2 * KT2, w)


def _make_in_maps(que, ans):
    """Normalize rows (folding the cosine norms into the quantization scale),
    quantize to fp8e4m3, and pack into the on-chip tile layouts. Also returns
    the exact host-computed diagonal logits."""
    fp8 = mybir.dt.np(FP8)
    que = np.asarray(que, dtype=np.float32)
    ans = np.asarray(ans, dtype=np.float32)

    qn = np.maximum(np.sqrt((que.astype(np.float64) ** 2).sum(1)), EPS)
    an = np.maximum(np.sqrt((ans.astype(np.float64) ** 2).sum(1)), EPS)
    q8 = (que * (SCALE / qn[:, None]).astype(np.float32)).astype(fp8)
    a8 = (ans * (SCALE / an[:, None]).astype(np.float32)).astype(fp8)

    # diag logits (exact, f64): cos(q_i, a_i) / gamma
    diag = (que.astype(np.float64) * ans.astype(np.float64)).sum(1) / (
        qn * an * GAMA
    )

    # column pieces (shared by all cores): grp0, grp1 halves, grp4 + 1MB grps
    aS = np.stack(
        [
            _pack_cols(a8, 0, 512),
            _pack_cols(a8, 512, 1024),
            _pack_cols(a8, 1024, 1536),
            _pack_cols(a8, 3584, 4096),
        ]
    )
    aF = np.stack([_pack_cols(a8, 1536, 2560), _pack_cols(a8, 2560, 3584)])

    in_maps = []
    for c in range(NCORES):
        qc = q8[c * NB : (c + 1) * NB]  # [512, 1024]
        # qPK[m, p, 2t+i, mm] = qc[128m+mm, 256t+128i+p]
        qPK = np.ascontiguousarray(
            qc.reshape(MT, P, KT2, 2, P).transpose(0, 4, 2, 3, 1)
        ).reshape(MT, P, 2 * KT2, P)
        in_maps.append({"qPK": qPK, "aS": aS, "aF": aF})
    return in_maps, diag


def _finish(results, diag):
    # s_out[p, 4*grp+m]: per-group partial softmax denominators.
    denoms = []
    for r in results:
        s = np.asarray(r["s_out"]).reshape(P, len(GW), MT).sum(axis=1)  # [p, m]
        denoms.append(s.T.reshape(-1))  # local row order m*128+p
    denom = np.concatenate(denoms)  # [B]
    lse = np.log(denom.astype(np.float64))
    loss = np.float32(np.mean(lse - diag))
    return np.array([loss], dtype=np.float32)


def kernel(que_batch, ans_batch):
    nc = _get_program()
    in_maps, diag = _make_in_maps(np.asarray(que_batch), np.asarray(ans_batch))
    res = run_bass_kernel_spmd(nc, in_maps, list(range(NCORES)))
    return _finish(res.results, diag)


if __name__ == "__main__":
    rng = np.random.default_rng(0)
    q = rng.standard_normal((B, D), dtype=np.float32)
    a = rng.standard_normal((B, D), dtype=np.float32)
    print(kernel(q, a))
